# revision 18
# baseline (speedup 1.0000x reference)
"""MLA (multi-head latent attention) forward on 8 TRN2 NeuronCores.

Sharding: core i owns heads {2i, 2i+1} for BOTH batches; compression is
sharded by (batch, window): core i compresses batch i//4, 512-token window
i%4.  Both exchanges are 8-core mesh collectives (4-core groups don't
support mesh/AllToAll):
 - kv latents: AllGather [128,3,512] -> [8,128,3,512] (both batches; ckv
   ships pre-scaled by the per-token rmsnorm rsqrt, kr rides in slot 2).
 - q: each core decompresses ALL 16 heads of its own tokens, applies
   rmsnorm+rope locally, then AllToAll routes head-pair chunks to their
   owner (1.4MB wire/core vs 2.4MB for gathering the cq latent).
Each core then decompresses k/v for its 2 heads (both batches), runs causal
attention, and computes the full out-proj partial for both batches; the host
sums the 8 partials per batch.

All matmuls bf16 (fp32 PSUM).  RMSNorm gains and the RoPE butterfly (sin==cos
bug preserved) are folded into the weights on the host; per-token rsqrt
factors and the cos table are applied as elementwise multiplies at
PSUM-eviction time.  Softmax skips the max subtraction (logits are O(10)) and
gets its denominator from an appended ones-column in V.

Perf structure:
 - DMA-issue cost on the issuing engine is ~5ns per descriptor LINE, so all
   bulk tensors are host-packed partition-major ([128, X] with fat contiguous
   lines) and moved by a handful of wide DMAs, split across the two hardware
   DGE queues (sync + scalar) so x/wcq stream in parallel.
 - compress accumulates per x-c-tile (PE starts on the first x chunk);
   m-tiles are packed in pairs into [128,1024] PSUM tiles so pool rotation
   never bubbles; the kv gather triggers before cq compression starts; a
   dummy warm-up collective at t=0 absorbs the ~30us first-collective
   barrier under the input DMA.
 - attention is software-pipelined: scores(kc) issue 2 chunks ahead of
   PV(kc-2) with exp on ACT in between, so the PE never waits on ACT; proj
   matmuls of the previous window are interleaved into the chunk loop, and
   proj PSUM evictions rotate across vector/gpsimd/scalar so no single
   engine's eviction load gates the PE.
Diagonal key chunks skip fully-masked query columns in scores/exp/PV;
softmax denominators use the ~5x-faster approximate reciprocal (ones column
sits at V slot 0 / PSUM partition 0 where it is valid); RMSNorm
partition-sums run on the PE.
"""

import sys

sys.path.insert(0, "/opt/trn_rl_repo")

import numpy as np
import ml_dtypes

from concourse import bacc, bass, bass_isa, mybir, tile
from concourse.bass_utils import run_bass_kernel_spmd

# problem dims (hardcoded per contract)
B, S, D = 2, 2048, 2048
H = 16
NOPE, ROPE, VD = 64, 32, 64
QR, KVR = 768, 256
EPS = 1e-6
THETA = 10000.0

HO = 2  # heads per core
NCORES = 8
P = 128
W = 512  # token window
NW = S // W  # 4
NC = D // P  # 16 contraction chunks
QKD = NOPE + ROPE  # 96

BF = mybir.dt.bfloat16
F32 = mybir.dt.float32
NBF = ml_dtypes.bfloat16
MULT = mybir.AluOpType.mult
AFT = mybir.ActivationFunctionType

GROUPS8 = [[0, 1, 2, 3, 4, 5, 6, 7]]
BW = [(b, w) for b in range(B) for w in range(NW)]  # window order

LAST_RESULT = None
_CACHE = {}


def _pack_pm(a):
    """[n*128, X] row-major -> [128, n*X] partition-major (fat DMA lines)."""
    n = a.shape[0] // P
    return np.ascontiguousarray(
        a.reshape(n, P, a.shape[1]).transpose(1, 0, 2).reshape(P, -1)
    )


def _build_nc():
    nc = bacc.Bacc("TRN2", debug=False)
    with tile.TileContext(nc) as tc:
        with (
            tc.tile_pool(name="dram", bufs=1, space="DRAM") as dram,
            tc.tile_pool(name="wres", bufs=1) as wres,
            tc.tile_pool(name="acts", bufs=1) as acts,
            tc.tile_pool(name="sq", bufs=2) as sqp,
            tc.tile_pool(name="pt", bufs=4) as ptp,
            tc.tile_pool(name="stage", bufs=6) as stg,
            tc.tile_pool(name="bc", bufs=3) as bcp,
            # one PSUM pool, three tags; 2*4KB + 2*2KB + 2*2KB = 16KB exact
            tc.tile_pool(name="ps", bufs=2, space="PSUM") as psp,
        ):
            # ------------- DRAM params (host-packed partition-major) -------------
            xTw = dram.tile([P, NC * W], BF, kind="ExternalInput", name="xTw",
                            uniquify=False)
            wcq = dram.tile([P, NC * QR], BF, kind="ExternalInput", name="wcq",
                            uniquify=False)
            wckvkr = dram.tile([P, NC * (KVR + ROPE)], BF, kind="ExternalInput",
                               name="wckvkr", uniquify=False)
            wq = dram.tile([P, (QR // P) * H * QKD], BF, kind="ExternalInput",
                           name="wq", uniquify=False)
            wkv = dram.tile([P, (KVR // P) * HO * (NOPE + VD)], BF,
                            kind="ExternalInput", name="wkv", uniquify=False)
            wproj = dram.tile([HO * VD, D], BF, kind="ExternalInput", name="wproj",
                              uniquify=False)
            cropew_d = dram.tile([ROPE, W], BF, kind="ExternalInput",
                                 name="cropew", uniquify=False)
            cropeq_d = dram.tile([P, W], BF, kind="ExternalInput", name="cropeq",
                                 uniquify=False)
            masks_d = dram.tile([P, P], BF, kind="ExternalInput", name="masks",
                                uniquify=False)
            # bf16 partials: host upcasts and sums in f32; rounding adds
            # ~1e-3 rel err against a 14e-3 margin, and halves the 32MB
            # output write + eviction cost
            out_d = dram.tile([B, S, D], BF, kind="ExternalOutput", name="out",
                              uniquify=False)

            # ---------------- resident SBUF ----------------
            x_sb = wres.tile([P, NC, W], BF, tag="x")
            wcq_sb = wres.tile([P, NC, QR], BF, tag="wcq")
            wckvkr_sb = wres.tile([P, NC, KVR + ROPE], BF, tag="wckvkr")
            wq_sb = wres.tile([P, QR // P, H * QKD], BF, tag="wq")
            wkv_sb = wres.tile([P, KVR // P, HO * (NOPE + VD)], BF, tag="wkv")
            wproj_sb = wres.tile([P, D], BF, tag="wproj")
            cropew_sb = wres.tile([ROPE, W], BF, tag="cropew")
            cropeq_sb = wres.tile([P, W], BF, tag="cropeq")
            masks_sb = wres.tile([P, P], BF, tag="masks")
            cb_sb = wres.tile([P, 4], F32, tag="cb")  # [sc_q, b_q, sc_kv, b_kv]
            ones_sb = wres.tile([P, 1], F32, tag="ones")
            warm_sb = wres.tile([8, 16], BF, tag="warm")

            nc.vector.memset(ones_sb[:], 1.0)
            nc.vector.memset(cb_sb[:, 0:1], float(QKD) / QR)
            nc.vector.memset(cb_sb[:, 1:2], float(QKD) * EPS)
            nc.vector.memset(cb_sb[:, 2:3], 1.0 / KVR)
            nc.vector.memset(cb_sb[:, 3:4], EPS)
            nc.vector.memset(warm_sb[:], 0.0)

            # ---------------- per-(batch,window) activations ----------------
            def bwtiles(shape, dt, base, pool=acts):
                return {
                    (b, w): pool.tile(
                        shape, dt, tag=f"{base}{b}{w}", name=f"{base}{b}{w}"
                    )
                    for (b, w) in BW
                }

            cqT_sb = acts.tile([P, QR // P, W], BF, tag="cqT", name="cqT")
            # slots 0,1 = pre-normed ckv; slot 2 rows 0:32 = kr (rest junk)
            ckvT = bwtiles([P, 3, W], BF, "ckvT")
            rqbc = acts.tile([P, W], F32, tag="rqbc", name="rqbc")
            # per-head V block is [96]: ones col at 0 (softmax denominator
            # lands on PSUM partition 0 where reciprocal_approx_fast works),
            # V at cols 32:96 (partition bases must be multiples of 32)
            vaug = bwtiles([P, NW, HO, 32 + VD], BF, "vaug")
            oT = bwtiles([P, W], BF, "oT")
            qT = bwtiles([QKD, HO, W], BF, "qT")
            kT = {
                (h, b, w): acts.tile(
                    [QKD, W], BF, tag=f"kT{h}_{b}{w}", name=f"kT{h}_{b}{w}"
                )
                for h in range(HO)
                for (b, w) in BW
            }

            # ---------------- DRAM collective buffers ----------------
            cc_in_a = dram.tile([P, 3, W], BF, kind="Internal", name="cc_in_a",
                                uniquify=False)
            cc_out_a = dram.tile([NCORES, P, 3, W], BF, kind="Internal",
                                 name="cc_out_a", uniquify=False,
                                 addr_space="Shared")
            cc_in_q = dram.tile([NCORES, QKD, HO, W], BF, kind="Internal",
                                name="cc_in_q", uniquify=False)
            cc_out_q = dram.tile([NCORES, QKD, HO, W], BF, kind="Internal",
                                 name="cc_out_q", uniquify=False)
            warm_in = dram.tile([8, 16], BF, kind="Internal", name="warm_in",
                                uniquify=False)
            warm_out = dram.tile([NCORES, 8, 16], BF, kind="Internal",
                                 name="warm_out", uniquify=False,
                                 addr_space="Shared")

            # =========================================================
            # DMA issue order = queue priority.  sync (SP) and scalar (ACT)
            # are independent hardware DGE queues; x+wckvkr+wq go on sync
            # while wcq+the rest stream on scalar in parallel.
            # =========================================================
            # warm-up collective first (absorbs the first-collective barrier)
            nc.sync.dma_start(out=warm_in[:], in_=warm_sb[:])
            nc.gpsimd.collective_compute(
                "AllGather",
                mybir.AluOpType.bypass,
                replica_groups=GROUPS8,
                ins=[warm_in[:]],
                outs=[warm_out[:]],
            )
            # x in 4 fat chunks; wckvkr right after the first so the kv
            # c-loop can start while the rest of x streams
            nc.sync.dma_start(out=x_sb[:, 0:4, :], in_=xTw[:, 0 : 4 * W])
            nc.sync.dma_start(out=wckvkr_sb[:], in_=wckvkr[:])
            for cg in range(1, 4):
                cs = slice(cg * 4 * W, (cg + 1) * 4 * W)
                nc.sync.dma_start(out=x_sb[:, 4 * cg : 4 * (cg + 1), :], in_=xTw[:, cs])
            nc.sync.dma_start(out=wq_sb[:], in_=wq[:])
            nc.sync.dma_start(out=wkv_sb[:], in_=wkv[:])
            nc.sync.dma_start(out=masks_sb[:], in_=masks_d[:])
            # scalar queue: wcq streams in parallel with x
            nc.scalar.dma_start(out=wcq_sb[:], in_=wcq[:])
            nc.scalar.dma_start(out=cropew_sb[:], in_=cropew_d[:])
            nc.scalar.dma_start(out=cropeq_sb[:], in_=cropeq_d[:])
            nc.scalar.dma_start(out=wproj_sb[:], in_=wproj[:])

            # ones columns of vaug (vector, no deps)
            for bw in BW:
                nc.vector.memset(vaug[bw][:, :, :, 0:1], 1.0)

            # =========================================================
            # PHASE C: compress own window, per-c accumulation.
            # Groups of two 128-row m-tiles share one [128,1024] PSUM tile
            # (disjoint column halves), so the 2-buf "sp" rotation always
            # reuses a tile whose eviction finished a full group ago.
            # =========================================================
            _sid = nc.enter_named_scope("cmp", False)[0]
            acc_kv = bcp.tile([P, W], F32, tag="acc", bufs=2)
            acc_q = bcp.tile([P, W], F32, tag="acc", bufs=2)

            def cpass(ws, m_tiles, psum, acc, first_m):
                for c in range(NC):
                    for j, m in enumerate(m_tiles):
                        nc.tensor.matmul(
                            psum[:, j * W : (j + 1) * W],
                            ws[:, c, m * P : (m + 1) * P],
                            x_sb[:, c, :],
                            start=(c == 0),
                            stop=(c == NC - 1),
                        )
                for j, m in enumerate(m_tiles):
                    sq = sqp.tile([P, W], BF, tag="sq")
                    nc.scalar.square(out=sq[:], in_=psum[:, j * W : (j + 1) * W])
                    if first_m and j == 0:
                        nc.vector.tensor_copy(out=acc[:], in_=sq[:])
                    else:
                        nc.vector.tensor_add(out=acc[:], in0=acc[:], in1=sq[:])

            # -- kv pass: ckv m0|m1, then kr --
            pkv = psp.tile([P, 2 * W], F32, tag="sp")
            cpass(wckvkr_sb, [0, 1], pkv, acc_kv, True)
            # rkv = rsqrt(mean+eps) row; partition-sum on the PE; ckv ships
            # pre-scaled so receivers need no per-token factor
            rps = psp.tile([1, W], F32, tag="pj")
            nc.tensor.matmul(rps[:], ones_sb[:], acc_kv[:], start=True, stop=True)
            t4 = bcp.tile([P, W], F32, tag="tmp2", bufs=2)
            nc.scalar.activation(
                out=t4[0:1, :], in_=rps[0:1, :], func=AFT.Sqrt,
                bias=cb_sb[0:1, 3:4], scale=cb_sb[0:1, 2:3],
            )
            rowkv = bcp.tile([1, W], F32, tag="row", bufs=2)
            nc.vector.reciprocal_approx_fast(out=rowkv[:], in_=t4[0:1, :])
            rkvb = bcp.tile([P, W], F32, tag="recb", bufs=2)
            nc.gpsimd.partition_broadcast(rkvb[:], rowkv[:])
            for j in range(2):
                st = stg.tile([P, W], BF, tag="st")
                nc.vector.tensor_tensor(
                    out=st[:], in0=pkv[:, j * W : (j + 1) * W], in1=rkvb[:], op=MULT
                )
                nc.sync.dma_start(out=cc_in_a[:, j, :], in_=st[:])
            pkr = psp.tile([ROPE, W], F32, tag="pj")
            for c in range(NC):
                nc.tensor.matmul(
                    pkr[:],
                    wckvkr_sb[:, c, KVR : KVR + ROPE],
                    x_sb[:, c, :],
                    start=(c == 0),
                    stop=(c == NC - 1),
                )
            st = stg.tile([ROPE, W], BF, tag="st")
            nc.vector.tensor_tensor(out=st[:], in0=pkr[:], in1=cropew_sb[:], op=MULT)
            nc.sync.dma_start(out=cc_in_a[0:ROPE, 2, :], in_=st[:])
            # kv-latent gather fires before cq compression even starts
            nc.gpsimd.collective_compute(
                "AllGather",
                mybir.AluOpType.bypass,
                replica_groups=GROUPS8,
                ins=[cc_in_a[:]],
                outs=[cc_out_a[:]],
            )

            # -- cq passes: 6 m-tiles in 3 groups; evict into local cqT --
            for gi in range(3):
                pq = psp.tile([P, 2 * W], F32, tag="sp")
                cpass(wcq_sb, [2 * gi, 2 * gi + 1], pq, acc_q, gi == 0)
                for j in range(2):
                    nc.scalar.copy(
                        out=cqT_sb[:, 2 * gi + j, :], in_=pq[:, j * W : (j + 1) * W]
                    )
            # rq = rsqrt(96*mean+96*eps) row (folds 1/sqrt(96) score scale)
            rps = psp.tile([1, W], F32, tag="pj")
            nc.tensor.matmul(rps[:], ones_sb[:], acc_q[:], start=True, stop=True)
            t2 = bcp.tile([P, W], F32, tag="tmp2", bufs=2)
            nc.scalar.activation(
                out=t2[0:1, :], in_=rps[0:1, :], func=AFT.Sqrt,
                bias=cb_sb[0:1, 1:2], scale=cb_sb[0:1, 0:1],
            )
            rowq = bcp.tile([1, W], F32, tag="row", bufs=2)
            nc.vector.reciprocal_approx_fast(out=rowq[:], in_=t2[0:1, :])
            nc.gpsimd.partition_broadcast(rqbc[:], rowq[:])
            nc.leave_named_scope("cmp", _sid, False)

            # =========================================================
            # PHASE DQ: decompress q for ALL 16 heads of own tokens, apply
            # rmsnorm + rope locally, ship head-pair chunks via AllToAll
            # (chunk j -> core j, which owns heads {2j, 2j+1}).
            # =========================================================
            _sid = nc.enter_named_scope("dq", False)[0]
            # crope has 4 stacked 32-row copies -> one [128,W] product
            # serves all heads' rope epilogues.
            crq = bcp.tile([P, W], BF, tag="crq", bufs=1)
            nc.vector.tensor_tensor(
                out=crq[:], in0=cropeq_sb[:], in1=rqbc[:], op=MULT
            )
            PTAGS = ["sp", "pj", "ot"]
            stq = None
            for h in range(H):
                psum = psp.tile([QKD, W], F32, tag=PTAGS[h % 3])
                for r in range(QR // P):
                    nc.tensor.matmul(
                        psum[:],
                        wq_sb[:, r, h * QKD : (h + 1) * QKD],
                        cqT_sb[:, r, :],
                        start=(r == 0),
                        stop=(r == QR // P - 1),
                    )
                if h % 2 == 0:
                    stq = stg.tile([QKD, HO, W], BF, tag="stq", bufs=3)
                nc.vector.tensor_tensor(
                    out=stq[0:NOPE, h % 2, :], in0=psum[0:NOPE, :],
                    in1=rqbc[0:NOPE, :], op=MULT,
                )
                nc.vector.tensor_tensor(
                    out=stq[NOPE:QKD, h % 2, :], in0=psum[NOPE:QKD, :],
                    in1=crq[ROPE * (h % 4) : ROPE * (h % 4 + 1), :], op=MULT,
                )
                if h % 2 == 1:
                    if h % 4 == 1:
                        nc.sync.dma_start(out=cc_in_q[h // 2], in_=stq[:])
                    else:
                        nc.scalar.dma_start(out=cc_in_q[h // 2], in_=stq[:])
            nc.gpsimd.collective_compute(
                "AllToAll",
                mybir.AluOpType.bypass,
                replica_groups=GROUPS8,
                ins=[cc_in_q[:]],
                outs=[cc_out_q[:]],
            )
            nc.leave_named_scope("dq", _sid, False)

            # ---- fills from the gathered latents / routed q ----
            _sid = nc.enter_named_scope("fill", False)[0]
            for src in range(NCORES):
                bw = (src // NW, src % NW)
                nc.sync.dma_start(out=ckvT[bw][:], in_=cc_out_a[src])
                nc.scalar.dma_start(out=qT[bw][:], in_=cc_out_q[src])
            nc.leave_named_scope("fill", _sid, False)

            # =========================================================
            # PHASE DKV: decompress k/v for own 2 heads, both batches
            # (runs under the AllToAll transfer).
            # =========================================================
            _sid = nc.enter_named_scope("dkv", False)[0]
            for i, bw in enumerate(BW):
                # k_nope: both heads in one [128,512] psum
                psum = psp.tile([P, W], F32, tag=PTAGS[i % 2])
                for r in range(KVR // P):
                    nc.tensor.matmul(
                        psum[:],
                        wkv_sb[:, r, 0 : HO * NOPE],
                        ckvT[bw][:, r, :],
                        start=(r == 0),
                        stop=(r == KVR // P - 1),
                    )
                for h in range(HO):
                    nc.vector.tensor_copy(
                        out=kT[(h,) + bw][0:NOPE, :],
                        in_=psum[NOPE * h : NOPE * (h + 1), :],
                    )
                    nc.vector.tensor_copy(
                        out=kT[(h,) + bw][NOPE:QKD, :], in_=ckvT[bw][0:ROPE, 2, :]
                    )
                # v (token-major; latent arrives pre-normed)
                for cc in range(NW):
                    psum = psp.tile([P, HO * VD], F32, tag=PTAGS[(i + cc) % 2])
                    for r in range(KVR // P):
                        nc.tensor.matmul(
                            psum[:],
                            ckvT[bw][:, r, cc * P : (cc + 1) * P],
                            wkv_sb[:, r, HO * NOPE : HO * (NOPE + VD)],
                            start=(r == 0),
                            stop=(r == KVR // P - 1),
                        )
                    nc.scalar.activation(
                        out=vaug[bw][:, cc, :, 32 : 32 + VD],
                        in_=psum[:].rearrange("p (h d) -> p h d", h=HO),
                        func=AFT.Copy,
                    )
            nc.leave_named_scope("dkv", _sid, False)

            # =========================================================
            # PHASE ATTN + PROJ: per (batch, window), software-pipelined
            # chunk loop with previous window's proj matmuls interleaved
            # to keep the PE dense; proj evictions rotate across engines.
            # =========================================================
            def proj_unit(bw, tt, wc):
                b, w = bw
                t = NW * w + tt
                wcs = slice(wc * W, (wc + 1) * W)
                psum = psp.tile([P, W], F32, tag="pj")
                nc.tensor.matmul(
                    psum[:],
                    oT[bw][:, tt * P : (tt + 1) * P],
                    wproj_sb[:, wcs],
                    start=True,
                    stop=True,
                )
                st = stg.tile([P, W], BF, tag="st")
                if wc % 2 == 1:
                    nc.scalar.copy(out=st[:], in_=psum[:])
                else:
                    nc.vector.tensor_copy(out=st[:], in_=psum[:])
                nc.sync.dma_start(out=out_d[b, t * P : (t + 1) * P, wcs], in_=st[:])

            def attn(bw, punits):
                """Pipelined attention for (batch, window) bw; `punits` is a
                list of (pbw, tt, wc) proj units interleaved into the PE
                stream."""
                b, w = bw
                nkc = 4 * w + 4
                nsteps = nkc + 2
                op0 = psp.tile([32 + VD, W], F32, tag="ot")
                op1 = psp.tile([32 + VD, W], F32, tag="ot")
                pts = {}
                for kc in range(nsteps):
                    if kc < nkc:
                        wk, ck = divmod(kc, NW)
                        cs = slice(ck * P, (ck + 1) * P)
                        t = kc - 4 * w
                        qlo = t * P if t > 0 else 0
                        sp = psp.tile([P, 2 * W], F32, tag="sp")
                        nc.tensor.matmul(
                            sp[:, qlo:W],
                            kT[(0, b, wk)][:, cs],
                            qT[(b, w)][:, 0, qlo:W],
                            start=True,
                            stop=True,
                        )
                        nc.tensor.matmul(
                            sp[:, W + qlo : 2 * W],
                            kT[(1, b, wk)][:, cs],
                            qT[(b, w)][:, 1, qlo:W],
                            start=True,
                            stop=True,
                        )
                        pt = ptp.tile([P, 2 * W], BF, tag="pt")
                        nc.scalar.activation(
                            out=pt[:, qlo : 2 * W],
                            in_=sp[:, qlo : 2 * W],
                            func=AFT.Exp,
                        )
                        if t >= 0:
                            # SBUF-only, so the (otherwise idle) gpsimd
                            # cores take the diagonal mask multiplies
                            ds0 = slice(t * P, (t + 1) * P)
                            ds1 = slice(W + t * P, W + (t + 1) * P)
                            nc.gpsimd.tensor_mul(
                                out=pt[:, ds0], in0=pt[:, ds0], in1=masks_sb[:]
                            )
                            nc.gpsimd.tensor_mul(
                                out=pt[:, ds1], in0=pt[:, ds1], in1=masks_sb[:]
                            )
                        pts[kc] = (pt, qlo)
                    k2 = kc - 2
                    if k2 >= 0:
                        wk2, ck2 = divmod(k2, NW)
                        pt2, qlo2 = pts.pop(k2)
                        nc.tensor.matmul(
                            op0[:, qlo2:W],
                            vaug[(b, wk2)][:, ck2, 0, :],
                            pt2[:, qlo2:W],
                            start=(k2 == 0),
                            stop=(k2 == nkc - 1),
                            skip_group_check=True,
                        )
                        nc.tensor.matmul(
                            op1[:, qlo2:W],
                            vaug[(b, wk2)][:, ck2, 1, :],
                            pt2[:, W + qlo2 : 2 * W],
                            start=(k2 == 0),
                            stop=(k2 == nkc - 1),
                            skip_group_check=True,
                        )
                    # interleave proj units for a dense PE stream
                    lo = (len(punits) * kc) // nsteps
                    hi = (len(punits) * (kc + 1)) // nsteps
                    for pbw, tt, wc in punits[lo:hi]:
                        proj_unit(pbw, tt, wc)
                for j, op in ((0, op0), (1, op1)):
                    rec = bcp.tile([1, W], F32, tag="row", bufs=2)
                    nc.vector.reciprocal_approx_fast(out=rec[:], in_=op[0:1, :])
                    recb = bcp.tile([P, W], F32, tag="recb", bufs=2)
                    nc.gpsimd.partition_broadcast(recb[:], rec[:])
                    # two 32-partition halves: a 64-partition access may only
                    # start at partition 0 or 64, and op's V rows start at 32
                    ob = NOPE * j
                    for z in range(2):
                        nc.vector.tensor_tensor(
                            out=oT[bw][ob + 32 * z : ob + 32 * (z + 1), :],
                            in0=op[32 * (z + 1) : 32 * (z + 2), :],
                            in1=recb[32 * z : 32 * (z + 1), :],
                            op=MULT,
                        )

            # proj lags attention by TWO windows: its oT source is then long
            # past the recip->broadcast->mul eviction chain, so interleaved
            # proj matmuls never stall the PE at window transitions.
            _sid = nc.enter_named_scope("attn", False)[0]
            for i, bw in enumerate(BW):
                punits = (
                    [(BW[i - 2], tt, wc) for tt in range(NW) for wc in range(NW)]
                    if i > 1
                    else []
                )
                attn(bw, punits)
            nc.leave_named_scope("attn", _sid, False)
            _sid = nc.enter_named_scope("proj", False)[0]
            for pbw in (BW[-2], BW[-1]):
                for tt in range(NW):
                    for wc in range(NW):
                        proj_unit(pbw, tt, wc)
            nc.leave_named_scope("proj", _sid, False)

    nc.compile()
    return nc


def _rope_fold():
    """32x32 butterfly for RoPE with the reference's sin==cos bug."""
    Bm = np.zeros((ROPE, ROPE), np.float32)
    for j in range(ROPE // 2):
        Bm[2 * j, 2 * j] = 1.0
        Bm[2 * j, 2 * j + 1] = -1.0
        Bm[2 * j + 1, 2 * j] = 1.0
        Bm[2 * j + 1, 2 * j + 1] = 1.0
    return Bm


def _host_tables():
    freqs = 1.0 / (THETA ** (np.arange(0, ROPE, 2, dtype=np.float32) / ROPE))
    ang = np.outer(np.arange(S, dtype=np.float32), freqs)  # [S, 16]
    cos = np.cos(ang)  # [S, 16]
    crope32 = np.repeat(cos, 2, axis=1).T.copy()  # [32, S]
    crope = np.tile(crope32, (4, 1)).astype(NBF)  # [128, S]
    # [key, query] triangle for the diagonal 128x128 block
    masks = (np.arange(P)[None, :] >= np.arange(P)[:, None]).astype(np.float32)
    return crope, masks.astype(NBF)


def kernel(**inputs):
    global LAST_RESULT
    x = np.asarray(inputs["x"], np.float32)
    w_cq = np.asarray(inputs["w_cq"], np.float32)
    w_q_nope = np.asarray(inputs["w_q_nope"], np.float32)
    w_q_rope = np.asarray(inputs["w_q_rope"], np.float32)
    q_g = np.asarray(inputs["q_g"], np.float32)
    w_ckv = np.asarray(inputs["w_ckv"], np.float32)
    w_k_nope = np.asarray(inputs["w_k_nope"], np.float32)
    w_v = np.asarray(inputs["w_v"], np.float32)
    kv_g = np.asarray(inputs["kv_g"], np.float32)
    w_k_rope = np.asarray(inputs["w_k_rope"], np.float32)
    w_proj = np.asarray(inputs["w_proj"], np.float32)

    Bm = _rope_fold()
    crope, masks = _host_tables()

    wqn = w_q_nope * q_g[:, None]  # [QR, H*64]
    wqr = w_q_rope * q_g[:, None]  # [QR, H*32]
    wkn = w_k_nope * kv_g[:, None]  # [KVR, H*64]
    wv = w_v * kv_g[:, None]  # [KVR, H*64]
    wkr = (w_k_rope @ Bm.T) / H  # [D, 32]
    wckvkr = np.concatenate([w_ckv, wkr], axis=1).astype(np.float32)  # [D, 288]

    # full wq (all 16 heads, nope|rotated-rope interleaved per head) — the
    # same tensor on every core; chunk j feeds the AllToAll slot for core j
    # (owner of heads 2j, 2j+1)
    wq_cols = []
    for h in range(H):
        wq_cols.append(wqn[:, h * NOPE : (h + 1) * NOPE])
        wq_cols.append(wqr[:, h * ROPE : (h + 1) * ROPE] @ Bm.T)
    wq_full = np.concatenate(wq_cols, axis=1)  # [QR, 1536]

    wcq_pm = _pack_pm(w_cq).astype(NBF)
    wckvkr_pm = _pack_pm(wckvkr).astype(NBF)
    wq_pm = _pack_pm(wq_full).astype(NBF)

    if "nc" not in _CACHE:
        _CACHE["nc"] = _build_nc()
    nc = _CACHE["nc"]

    in_maps = []
    for core in range(NCORES):
        bo, wo = divmod(core, NW)
        heads = range(HO * core, HO * (core + 1))
        wkv_core = np.concatenate(
            [wkn[:, h * NOPE : (h + 1) * NOPE] for h in heads]
            + [wv[:, h * VD : (h + 1) * VD] for h in heads],
            axis=1,
        )  # [KVR, 256]
        wproj_core = np.concatenate(
            [w_proj[h * VD : (h + 1) * VD, :] for h in heads], axis=0
        )  # [128, D]
        ws = slice(W * wo, W * (wo + 1))
        in_maps.append(
            {
                "xTw": _pack_pm(np.ascontiguousarray(x[bo].T[:, ws])).astype(NBF),
                "cropew": np.ascontiguousarray(crope[0:ROPE, ws]),
                "cropeq": np.ascontiguousarray(crope[:, ws]),
                "wcq": wcq_pm,
                "wckvkr": wckvkr_pm,
                "wq": wq_pm,
                "wkv": _pack_pm(wkv_core).astype(NBF),
                "wproj": wproj_core.astype(NBF),
                "masks": masks,
            }
        )

    res = run_bass_kernel_spmd(nc, in_maps, list(range(NCORES)))
    LAST_RESULT = res
    outs = [np.asarray(r["out"], np.float32) for r in res.results]
    out = sum(outs)
    return out


# revision 21
# speedup vs baseline: 1.6336x; 1.6336x over previous
"""MLA (multi-head latent attention) forward on 8 TRN2 NeuronCores.

Sharding: core i owns heads {2i, 2i+1} for BOTH batches; compression is
sharded by (batch, window): core i compresses batch i//4, 512-token window
i%4.  Both exchanges are 8-core mesh collectives (4-core groups don't
support mesh/AllToAll):
 - kv latents: AllGather [128,3,512] -> [8,128,3,512] (both batches; ckv
   ships pre-scaled by the per-token rmsnorm rsqrt, kr rides in slot 2).
 - q: each core decompresses ALL 16 heads of its own tokens, applies
   rmsnorm+rope locally, then AllToAll routes head-pair chunks to their
   owner (1.4MB wire/core vs 2.4MB for gathering the cq latent).
Each core then decompresses k/v for its 2 heads (both batches), runs causal
attention, and computes the full out-proj partial for both batches; the host
sums the 8 partials per batch.

All matmuls bf16 (fp32 PSUM).  RMSNorm gains and the RoPE butterfly (sin==cos
bug preserved) are folded into the weights on the host; per-token rsqrt
factors and the cos table are applied as elementwise multiplies at
PSUM-eviction time.  Softmax skips the max subtraction (logits are O(10)) and
gets its denominator from an appended ones-column in V.

Perf structure:
 - DMA-issue cost on the issuing engine is ~5ns per descriptor LINE, so all
   bulk tensors are host-packed partition-major ([128, X] with fat contiguous
   lines) and moved by a handful of wide DMAs, split across the two hardware
   DGE queues (sync + scalar) so x/wcq stream in parallel.
 - compress accumulates per x-c-tile (PE starts on the first x chunk);
   m-tiles are packed in pairs into [128,1024] PSUM tiles so pool rotation
   never bubbles; the kv gather triggers before cq compression starts; a
   dummy warm-up collective at t=0 absorbs the ~30us first-collective
   barrier under the input DMA.
 - attention is software-pipelined: scores(kc) issue 2 chunks ahead of
   PV(kc-2) with exp on ACT in between, so the PE never waits on ACT; proj
   matmuls of the previous window are interleaved into the chunk loop, and
   proj PSUM evictions rotate across vector/gpsimd/scalar so no single
   engine's eviction load gates the PE.
Diagonal key chunks skip fully-masked query columns in scores/exp/PV;
softmax denominators use the ~5x-faster approximate reciprocal (ones column
sits at V slot 0 / PSUM partition 0 where it is valid); RMSNorm
partition-sums run on the PE.
"""

import sys

sys.path.insert(0, "/opt/trn_rl_repo")

import numpy as np
import ml_dtypes

from concourse import bacc, bass, bass_isa, mybir, tile
from concourse.bass_utils import run_bass_kernel_spmd

# problem dims (hardcoded per contract)
B, S, D = 2, 2048, 2048
H = 16
NOPE, ROPE, VD = 64, 32, 64
QR, KVR = 768, 256
EPS = 1e-6
THETA = 10000.0

HO = 2  # heads per core
NCORES = 8
P = 128
W = 512  # token window
NW = S // W  # 4
NC = D // P  # 16 contraction chunks
QKD = NOPE + ROPE  # 96

BF = mybir.dt.bfloat16
F32 = mybir.dt.float32
NBF = ml_dtypes.bfloat16
MULT = mybir.AluOpType.mult
AFT = mybir.ActivationFunctionType

GROUPS8 = [[0, 1, 2, 3, 4, 5, 6, 7]]
BW = [(b, w) for b in range(B) for w in range(NW)]  # window order

LAST_RESULT = None
_CACHE = {}


def _pack_pm(a):
    """[n*128, X] row-major -> [128, n*X] partition-major (fat DMA lines)."""
    n = a.shape[0] // P
    return np.ascontiguousarray(
        a.reshape(n, P, a.shape[1]).transpose(1, 0, 2).reshape(P, -1)
    )


def _build_nc():
    nc = bacc.Bacc("TRN2", debug=False)
    with tile.TileContext(nc) as tc:
        with (
            tc.tile_pool(name="dram", bufs=1, space="DRAM") as dram,
            tc.tile_pool(name="wres", bufs=1) as wres,
            tc.tile_pool(name="acts", bufs=1) as acts,
            tc.tile_pool(name="sq", bufs=2) as sqp,
            tc.tile_pool(name="pt", bufs=4) as ptp,
            tc.tile_pool(name="stage", bufs=6) as stg,
            tc.tile_pool(name="bc", bufs=3) as bcp,
            # one PSUM pool, three tags; 2*4KB + 2*2KB + 2*2KB = 16KB exact
            tc.tile_pool(name="ps", bufs=2, space="PSUM") as psp,
        ):
            # ------------- DRAM params (host-packed partition-major) -------------
            xTw = dram.tile([P, NC * W], BF, kind="ExternalInput", name="xTw",
                            uniquify=False)
            wcq = dram.tile([P, NC * QR], BF, kind="ExternalInput", name="wcq",
                            uniquify=False)
            wckvkr = dram.tile([P, NC * (KVR + ROPE)], BF, kind="ExternalInput",
                               name="wckvkr", uniquify=False)
            wq = dram.tile([P, (QR // P) * H * QKD], BF, kind="ExternalInput",
                           name="wq", uniquify=False)
            wkv = dram.tile([P, (KVR // P) * HO * (NOPE + VD)], BF,
                            kind="ExternalInput", name="wkv", uniquify=False)
            wproj = dram.tile([HO * VD, D], BF, kind="ExternalInput", name="wproj",
                              uniquify=False)
            cropew_d = dram.tile([ROPE, W], BF, kind="ExternalInput",
                                 name="cropew", uniquify=False)
            cropeq_d = dram.tile([P, W], BF, kind="ExternalInput", name="cropeq",
                                 uniquify=False)
            masks_d = dram.tile([P, P], BF, kind="ExternalInput", name="masks",
                                uniquify=False)
            # bf16 partials: host upcasts and sums in f32; rounding adds
            # ~1e-3 rel err against a 14e-3 margin, and halves the 32MB
            # output write + eviction cost
            out_d = dram.tile([B, S, D], BF, kind="ExternalOutput", name="out",
                              uniquify=False)

            # ---------------- resident SBUF ----------------
            x_sb = wres.tile([P, NC, W], BF, tag="x")
            wcq_sb = wres.tile([P, NC, QR], BF, tag="wcq")
            wckvkr_sb = wres.tile([P, NC, KVR + ROPE], BF, tag="wckvkr")
            wq_sb = wres.tile([P, QR // P, H * QKD], BF, tag="wq")
            wkv_sb = wres.tile([P, KVR // P, HO * (NOPE + VD)], BF, tag="wkv")
            wproj_sb = wres.tile([P, D], BF, tag="wproj")
            cropew_sb = wres.tile([ROPE, W], BF, tag="cropew")
            cropeq_sb = wres.tile([P, W], BF, tag="cropeq")
            masks_sb = wres.tile([P, P], BF, tag="masks")
            cb_sb = wres.tile([P, 4], F32, tag="cb")  # [sc_q, b_q, sc_kv, b_kv]
            ones_sb = wres.tile([P, 1], F32, tag="ones")
            warm_sb = wres.tile([8, 16], BF, tag="warm")

            nc.vector.memset(ones_sb[:], 1.0)
            nc.vector.memset(cb_sb[:, 0:1], float(QKD) / QR)
            nc.vector.memset(cb_sb[:, 1:2], float(QKD) * EPS)
            nc.vector.memset(cb_sb[:, 2:3], 1.0 / KVR)
            nc.vector.memset(cb_sb[:, 3:4], EPS)
            nc.vector.memset(warm_sb[:], 0.0)

            # ---------------- per-(batch,window) activations ----------------
            def bwtiles(shape, dt, base, pool=acts):
                return {
                    (b, w): pool.tile(
                        shape, dt, tag=f"{base}{b}{w}", name=f"{base}{b}{w}"
                    )
                    for (b, w) in BW
                }

            cqT_sb = acts.tile([P, QR // P, W], BF, tag="cqT", name="cqT")
            # slots 0,1 = pre-normed ckv; slot 2 rows 0:32 = kr (rest junk)
            ckvT = bwtiles([P, 3, W], BF, "ckvT")
            rqbc = acts.tile([P, W], F32, tag="rqbc", name="rqbc")
            # per-head V block is [96]: ones col at 0 (softmax denominator
            # lands on PSUM partition 0 where reciprocal_approx_fast works),
            # V at cols 32:96 (partition bases must be multiples of 32)
            vaug = bwtiles([P, NW, HO, 32 + VD], BF, "vaug")
            oT = bwtiles([P, W], BF, "oT")
            qT = bwtiles([QKD, HO, W], BF, "qT")
            kT = {
                (h, b, w): acts.tile(
                    [QKD, W], BF, tag=f"kT{h}_{b}{w}", name=f"kT{h}_{b}{w}"
                )
                for h in range(HO)
                for (b, w) in BW
            }

            # ---------------- DRAM collective buffers ----------------
            cc_in_a = dram.tile([P, 3, W], BF, kind="Internal", name="cc_in_a",
                                uniquify=False)
            cc_out_a = dram.tile([NCORES, P, 3, W], BF, kind="Internal",
                                 name="cc_out_a", uniquify=False,
                                 addr_space="Shared")
            cc_in_q = dram.tile([NCORES, QKD, HO, W], BF, kind="Internal",
                                name="cc_in_q", uniquify=False)
            cc_out_q = dram.tile([NCORES, QKD, HO, W], BF, kind="Internal",
                                 name="cc_out_q", uniquify=False)
            warm_in = dram.tile([8, 16], BF, kind="Internal", name="warm_in",
                                uniquify=False)
            warm_out = dram.tile([NCORES, 8, 16], BF, kind="Internal",
                                 name="warm_out", uniquify=False,
                                 addr_space="Shared")

            # =========================================================
            # DMA issue order = queue priority.  sync (SP) and scalar (ACT)
            # are independent hardware DGE queues; x+wckvkr+wq go on sync
            # while wcq+the rest stream on scalar in parallel.
            # =========================================================
            # warm-up collective first (absorbs the first-collective barrier)
            nc.sync.dma_start(out=warm_in[:], in_=warm_sb[:])
            nc.gpsimd.collective_compute(
                "AllGather",
                mybir.AluOpType.bypass,
                replica_groups=GROUPS8,
                ins=[warm_in[:]],
                outs=[warm_out[:]],
            )
            # x in 4 fat chunks; wckvkr right after the first so the kv
            # c-loop can start while the rest of x streams
            nc.sync.dma_start(out=x_sb[:, 0:4, :], in_=xTw[:, 0 : 4 * W])
            nc.sync.dma_start(
                out=wckvkr_sb[:, 0:8, :], in_=wckvkr[:, 0 : 8 * (KVR + ROPE)]
            )
            nc.sync.dma_start(
                out=wckvkr_sb[:, 8:16, :], in_=wckvkr[:, 8 * (KVR + ROPE) :]
            )
            for cg in range(1, 4):
                cs = slice(cg * 4 * W, (cg + 1) * 4 * W)
                nc.sync.dma_start(out=x_sb[:, 4 * cg : 4 * (cg + 1), :], in_=xTw[:, cs])
            nc.sync.dma_start(out=wq_sb[:], in_=wq[:])
            nc.sync.dma_start(out=wkv_sb[:], in_=wkv[:])
            nc.sync.dma_start(out=masks_sb[:], in_=masks_d[:])
            # scalar queue: wcq streams in parallel with x
            nc.scalar.dma_start(out=wcq_sb[:], in_=wcq[:])
            nc.scalar.dma_start(out=cropew_sb[:], in_=cropew_d[:])
            nc.scalar.dma_start(out=cropeq_sb[:], in_=cropeq_d[:])
            nc.scalar.dma_start(out=wproj_sb[:], in_=wproj[:])

            # ones columns of vaug (vector, no deps)
            for bw in BW:
                nc.vector.memset(vaug[bw][:, :, :, 0:1], 1.0)

            # =========================================================
            # PHASE C: compress own window, per-c accumulation.
            # Groups of two 128-row m-tiles share one [128,1024] PSUM tile
            # (disjoint column halves), so the 2-buf "sp" rotation always
            # reuses a tile whose eviction finished a full group ago.
            # =========================================================
            _sid = nc.enter_named_scope("cmp", False)[0]
            acc_kv = bcp.tile([P, W], F32, tag="acc", bufs=2)
            acc_q = bcp.tile([P, W], F32, tag="acc", bufs=2)

            def cpass(ws, m_tiles, psum, acc, first_m):
                for c in range(NC):
                    for j, m in enumerate(m_tiles):
                        nc.tensor.matmul(
                            psum[:, j * W : (j + 1) * W],
                            ws[:, c, m * P : (m + 1) * P],
                            x_sb[:, c, :],
                            start=(c == 0),
                            stop=(c == NC - 1),
                        )
                for j, m in enumerate(m_tiles):
                    sq = sqp.tile([P, W], BF, tag="sq")
                    nc.scalar.square(out=sq[:], in_=psum[:, j * W : (j + 1) * W])
                    if first_m and j == 0:
                        nc.vector.tensor_copy(out=acc[:], in_=sq[:])
                    else:
                        nc.vector.tensor_add(out=acc[:], in0=acc[:], in1=sq[:])

            # -- kv pass: ckv m0|m1, then kr --
            pkv = psp.tile([P, 2 * W], F32, tag="sp")
            cpass(wckvkr_sb, [0, 1], pkv, acc_kv, True)
            # rkv = rsqrt(mean+eps) row; partition-sum on the PE; ckv ships
            # pre-scaled so receivers need no per-token factor
            rps = psp.tile([1, W], F32, tag="pj")
            nc.tensor.matmul(rps[:], ones_sb[:], acc_kv[:], start=True, stop=True)
            t4 = bcp.tile([P, W], F32, tag="tmp2", bufs=2)
            nc.scalar.activation(
                out=t4[0:1, :], in_=rps[0:1, :], func=AFT.Sqrt,
                bias=cb_sb[0:1, 3:4], scale=cb_sb[0:1, 2:3],
            )
            rowkv = bcp.tile([1, W], F32, tag="row", bufs=2)
            nc.vector.reciprocal_approx_fast(out=rowkv[:], in_=t4[0:1, :])
            rkvb = bcp.tile([P, W], F32, tag="recb", bufs=2)
            nc.gpsimd.partition_broadcast(rkvb[:], rowkv[:])
            for j in range(2):
                st = stg.tile([P, W], BF, tag="st")
                nc.vector.tensor_tensor(
                    out=st[:], in0=pkv[:, j * W : (j + 1) * W], in1=rkvb[:], op=MULT
                )
                nc.sync.dma_start(out=cc_in_a[:, j, :], in_=st[:])
            pkr = psp.tile([ROPE, W], F32, tag="pj")
            for c in range(NC):
                nc.tensor.matmul(
                    pkr[:],
                    wckvkr_sb[:, c, KVR : KVR + ROPE],
                    x_sb[:, c, :],
                    start=(c == 0),
                    stop=(c == NC - 1),
                )
            st = stg.tile([ROPE, W], BF, tag="st")
            nc.vector.tensor_tensor(out=st[:], in0=pkr[:], in1=cropew_sb[:], op=MULT)
            nc.sync.dma_start(out=cc_in_a[0:ROPE, 2, :], in_=st[:])
            # kv-latent gather fires before cq compression even starts
            nc.gpsimd.collective_compute(
                "AllGather",
                mybir.AluOpType.bypass,
                replica_groups=GROUPS8,
                ins=[cc_in_a[:]],
                outs=[cc_out_a[:]],
            )

            # -- cq passes: 6 m-tiles in 3 groups; evict into local cqT --
            for gi in range(3):
                pq = psp.tile([P, 2 * W], F32, tag="sp")
                cpass(wcq_sb, [2 * gi, 2 * gi + 1], pq, acc_q, gi == 0)
                for j in range(2):
                    nc.scalar.copy(
                        out=cqT_sb[:, 2 * gi + j, :], in_=pq[:, j * W : (j + 1) * W]
                    )
            # rq = rsqrt(96*mean+96*eps) row (folds 1/sqrt(96) score scale)
            rps = psp.tile([1, W], F32, tag="pj")
            nc.tensor.matmul(rps[:], ones_sb[:], acc_q[:], start=True, stop=True)
            t2 = bcp.tile([P, W], F32, tag="tmp2", bufs=2)
            nc.scalar.activation(
                out=t2[0:1, :], in_=rps[0:1, :], func=AFT.Sqrt,
                bias=cb_sb[0:1, 1:2], scale=cb_sb[0:1, 0:1],
            )
            rowq = bcp.tile([1, W], F32, tag="row", bufs=2)
            nc.vector.reciprocal_approx_fast(out=rowq[:], in_=t2[0:1, :])
            nc.gpsimd.partition_broadcast(rqbc[:], rowq[:])
            nc.leave_named_scope("cmp", _sid, False)

            # =========================================================
            # PHASE DQ: decompress q for ALL 16 heads of own tokens, apply
            # rmsnorm + rope locally, ship head-pair chunks via AllToAll
            # (chunk j -> core j, which owns heads {2j, 2j+1}).
            # =========================================================
            _sid = nc.enter_named_scope("dq", False)[0]
            # crope has 4 stacked 32-row copies -> one [128,W] product
            # serves all heads' rope epilogues.
            crq = bcp.tile([P, W], BF, tag="crq", bufs=1)
            nc.vector.tensor_tensor(
                out=crq[:], in0=cropeq_sb[:], in1=rqbc[:], op=MULT
            )
            PTAGS = ["sp", "pj", "ot"]
            stq = None
            for h in range(H):
                psum = psp.tile([QKD, W], F32, tag=PTAGS[h % 3])
                for r in range(QR // P):
                    nc.tensor.matmul(
                        psum[:],
                        wq_sb[:, r, h * QKD : (h + 1) * QKD],
                        cqT_sb[:, r, :],
                        start=(r == 0),
                        stop=(r == QR // P - 1),
                    )
                if h % 2 == 0:
                    stq = stg.tile([QKD, HO, W], BF, tag="stq", bufs=3)
                nc.vector.tensor_tensor(
                    out=stq[0:NOPE, h % 2, :], in0=psum[0:NOPE, :],
                    in1=rqbc[0:NOPE, :], op=MULT,
                )
                nc.vector.tensor_tensor(
                    out=stq[NOPE:QKD, h % 2, :], in0=psum[NOPE:QKD, :],
                    in1=crq[ROPE * (h % 4) : ROPE * (h % 4 + 1), :], op=MULT,
                )
                if h % 2 == 1:
                    if h % 4 == 1:
                        nc.sync.dma_start(out=cc_in_q[h // 2], in_=stq[:])
                    else:
                        nc.scalar.dma_start(out=cc_in_q[h // 2], in_=stq[:])
            nc.gpsimd.collective_compute(
                "AllToAll",
                mybir.AluOpType.bypass,
                replica_groups=GROUPS8,
                ins=[cc_in_q[:]],
                outs=[cc_out_q[:]],
            )
            nc.leave_named_scope("dq", _sid, False)

            # ---- fills from the gathered latents / routed q ----
            # fills stay on the sync queue: a DMA issue that WAITS (on the
            # collective semaphores) blocks its whole engine queue, and the
            # ACT queue must keep flowing (dkv evictions, attention exps)
            _sid = nc.enter_named_scope("fill", False)[0]
            for src in range(NCORES):
                bw = (src // NW, src % NW)
                nc.sync.dma_start(out=ckvT[bw][:], in_=cc_out_a[src])
                nc.sync.dma_start(out=qT[bw][:], in_=cc_out_q[src])
            nc.leave_named_scope("fill", _sid, False)

            # =========================================================
            # PHASE DKV: decompress k/v for own 2 heads, both batches
            # (runs under the AllToAll transfer).
            # =========================================================
            _sid = nc.enter_named_scope("dkv", False)[0]
            for i, bw in enumerate(BW):
                # k_nope: both heads in one [128,512] psum
                psum = psp.tile([P, W], F32, tag=PTAGS[i % 2])
                for r in range(KVR // P):
                    nc.tensor.matmul(
                        psum[:],
                        wkv_sb[:, r, 0 : HO * NOPE],
                        ckvT[bw][:, r, :],
                        start=(r == 0),
                        stop=(r == KVR // P - 1),
                    )
                for h in range(HO):
                    nc.vector.tensor_copy(
                        out=kT[(h,) + bw][0:NOPE, :],
                        in_=psum[NOPE * h : NOPE * (h + 1), :],
                    )
                    nc.vector.tensor_copy(
                        out=kT[(h,) + bw][NOPE:QKD, :], in_=ckvT[bw][0:ROPE, 2, :]
                    )
                # v (token-major; latent arrives pre-normed)
                for cc in range(NW):
                    psum = psp.tile([P, HO * VD], F32, tag=PTAGS[(i + cc) % 2])
                    for r in range(KVR // P):
                        nc.tensor.matmul(
                            psum[:],
                            ckvT[bw][:, r, cc * P : (cc + 1) * P],
                            wkv_sb[:, r, HO * NOPE : HO * (NOPE + VD)],
                            start=(r == 0),
                            stop=(r == KVR // P - 1),
                        )
                    nc.scalar.activation(
                        out=vaug[bw][:, cc, :, 32 : 32 + VD],
                        in_=psum[:].rearrange("p (h d) -> p h d", h=HO),
                        func=AFT.Copy,
                    )
            nc.leave_named_scope("dkv", _sid, False)

            # =========================================================
            # PHASE ATTN + PROJ: per (batch, window), software-pipelined
            # chunk loop with previous window's proj matmuls interleaved
            # to keep the PE dense; proj evictions rotate across engines.
            # =========================================================
            def proj_unit(bw, tt, wc):
                b, w = bw
                t = NW * w + tt
                wcs = slice(wc * W, (wc + 1) * W)
                psum = psp.tile([P, W], F32, tag="pj")
                nc.tensor.matmul(
                    psum[:],
                    oT[bw][:, tt * P : (tt + 1) * P],
                    wproj_sb[:, wcs],
                    start=True,
                    stop=True,
                )
                st = stg.tile([P, W], BF, tag="st")
                if wc % 2 == 1:
                    nc.scalar.copy(out=st[:], in_=psum[:])
                else:
                    nc.vector.tensor_copy(out=st[:], in_=psum[:])
                nc.sync.dma_start(out=out_d[b, t * P : (t + 1) * P, wcs], in_=st[:])

            def attn(bw, punits):
                """Pipelined attention for (batch, window) bw; `punits` is a
                list of (pbw, tt, wc) proj units interleaved into the PE
                stream."""
                b, w = bw
                nkc = 4 * w + 4
                nsteps = nkc + 2
                op0 = psp.tile([32 + VD, W], F32, tag="ot")
                op1 = psp.tile([32 + VD, W], F32, tag="ot")
                pts = {}
                for kc in range(nsteps):
                    if kc < nkc:
                        wk, ck = divmod(kc, NW)
                        cs = slice(ck * P, (ck + 1) * P)
                        t = kc - 4 * w
                        qlo = t * P if t > 0 else 0
                        sp = psp.tile([P, 2 * W], F32, tag="sp")
                        nc.tensor.matmul(
                            sp[:, qlo:W],
                            kT[(0, b, wk)][:, cs],
                            qT[(b, w)][:, 0, qlo:W],
                            start=True,
                            stop=True,
                        )
                        nc.tensor.matmul(
                            sp[:, W + qlo : 2 * W],
                            kT[(1, b, wk)][:, cs],
                            qT[(b, w)][:, 1, qlo:W],
                            start=True,
                            stop=True,
                        )
                        pt = ptp.tile([P, 2 * W], BF, tag="pt")
                        nc.scalar.activation(
                            out=pt[:, qlo : 2 * W],
                            in_=sp[:, qlo : 2 * W],
                            func=AFT.Exp,
                        )
                        if t >= 0:
                            # NOT gpsimd: alternating gpsimd op types pays a
                            # ~6us ucode library swap against the broadcasts
                            ds0 = slice(t * P, (t + 1) * P)
                            ds1 = slice(W + t * P, W + (t + 1) * P)
                            nc.vector.tensor_mul(
                                out=pt[:, ds0], in0=pt[:, ds0], in1=masks_sb[:]
                            )
                            nc.vector.tensor_mul(
                                out=pt[:, ds1], in0=pt[:, ds1], in1=masks_sb[:]
                            )
                        pts[kc] = (pt, qlo)
                    k2 = kc - 2
                    if k2 >= 0:
                        wk2, ck2 = divmod(k2, NW)
                        pt2, qlo2 = pts.pop(k2)
                        nc.tensor.matmul(
                            op0[:, qlo2:W],
                            vaug[(b, wk2)][:, ck2, 0, :],
                            pt2[:, qlo2:W],
                            start=(k2 == 0),
                            stop=(k2 == nkc - 1),
                            skip_group_check=True,
                        )
                        nc.tensor.matmul(
                            op1[:, qlo2:W],
                            vaug[(b, wk2)][:, ck2, 1, :],
                            pt2[:, W + qlo2 : 2 * W],
                            start=(k2 == 0),
                            stop=(k2 == nkc - 1),
                            skip_group_check=True,
                        )
                    # interleave proj units for a dense PE stream
                    lo = (len(punits) * kc) // nsteps
                    hi = (len(punits) * (kc + 1)) // nsteps
                    for pbw, tt, wc in punits[lo:hi]:
                        proj_unit(pbw, tt, wc)
                for j, op in ((0, op0), (1, op1)):
                    rec = bcp.tile([1, W], F32, tag="row", bufs=2)
                    nc.vector.reciprocal_approx_fast(out=rec[:], in_=op[0:1, :])
                    recb = bcp.tile([P, W], F32, tag="recb", bufs=2)
                    nc.gpsimd.partition_broadcast(recb[:], rec[:])
                    # two 32-partition halves: a 64-partition access may only
                    # start at partition 0 or 64, and op's V rows start at 32
                    ob = NOPE * j
                    for z in range(2):
                        nc.vector.tensor_tensor(
                            out=oT[bw][ob + 32 * z : ob + 32 * (z + 1), :],
                            in0=op[32 * (z + 1) : 32 * (z + 2), :],
                            in1=recb[32 * z : 32 * (z + 1), :],
                            op=MULT,
                        )

            # proj lags attention by TWO windows: its oT source is then long
            # past the recip->broadcast->mul eviction chain, so interleaved
            # proj matmuls never stall the PE at window transitions.
            _sid = nc.enter_named_scope("attn", False)[0]
            for i, bw in enumerate(BW):
                punits = (
                    [(BW[i - 2], tt, wc) for tt in range(NW) for wc in range(NW)]
                    if i > 1
                    else []
                )
                attn(bw, punits)
            nc.leave_named_scope("attn", _sid, False)
            _sid = nc.enter_named_scope("proj", False)[0]
            for pbw in (BW[-2], BW[-1]):
                for tt in range(NW):
                    for wc in range(NW):
                        proj_unit(pbw, tt, wc)
            nc.leave_named_scope("proj", _sid, False)

    nc.compile()
    return nc


def _rope_fold():
    """32x32 butterfly for RoPE with the reference's sin==cos bug."""
    Bm = np.zeros((ROPE, ROPE), np.float32)
    for j in range(ROPE // 2):
        Bm[2 * j, 2 * j] = 1.0
        Bm[2 * j, 2 * j + 1] = -1.0
        Bm[2 * j + 1, 2 * j] = 1.0
        Bm[2 * j + 1, 2 * j + 1] = 1.0
    return Bm


def _host_tables():
    freqs = 1.0 / (THETA ** (np.arange(0, ROPE, 2, dtype=np.float32) / ROPE))
    ang = np.outer(np.arange(S, dtype=np.float32), freqs)  # [S, 16]
    cos = np.cos(ang)  # [S, 16]
    crope32 = np.repeat(cos, 2, axis=1).T.copy()  # [32, S]
    crope = np.tile(crope32, (4, 1)).astype(NBF)  # [128, S]
    # [key, query] triangle for the diagonal 128x128 block
    masks = (np.arange(P)[None, :] >= np.arange(P)[:, None]).astype(np.float32)
    return crope, masks.astype(NBF)


def kernel(**inputs):
    global LAST_RESULT
    x = np.asarray(inputs["x"], np.float32)
    w_cq = np.asarray(inputs["w_cq"], np.float32)
    w_q_nope = np.asarray(inputs["w_q_nope"], np.float32)
    w_q_rope = np.asarray(inputs["w_q_rope"], np.float32)
    q_g = np.asarray(inputs["q_g"], np.float32)
    w_ckv = np.asarray(inputs["w_ckv"], np.float32)
    w_k_nope = np.asarray(inputs["w_k_nope"], np.float32)
    w_v = np.asarray(inputs["w_v"], np.float32)
    kv_g = np.asarray(inputs["kv_g"], np.float32)
    w_k_rope = np.asarray(inputs["w_k_rope"], np.float32)
    w_proj = np.asarray(inputs["w_proj"], np.float32)

    Bm = _rope_fold()
    crope, masks = _host_tables()

    wqn = w_q_nope * q_g[:, None]  # [QR, H*64]
    wqr = w_q_rope * q_g[:, None]  # [QR, H*32]
    wkn = w_k_nope * kv_g[:, None]  # [KVR, H*64]
    wv = w_v * kv_g[:, None]  # [KVR, H*64]
    wkr = (w_k_rope @ Bm.T) / H  # [D, 32]
    wckvkr = np.concatenate([w_ckv, wkr], axis=1).astype(np.float32)  # [D, 288]

    # full wq (all 16 heads, nope|rotated-rope interleaved per head) — the
    # same tensor on every core; chunk j feeds the AllToAll slot for core j
    # (owner of heads 2j, 2j+1)
    wq_cols = []
    for h in range(H):
        wq_cols.append(wqn[:, h * NOPE : (h + 1) * NOPE])
        wq_cols.append(wqr[:, h * ROPE : (h + 1) * ROPE] @ Bm.T)
    wq_full = np.concatenate(wq_cols, axis=1)  # [QR, 1536]

    wcq_pm = _pack_pm(w_cq).astype(NBF)
    wckvkr_pm = _pack_pm(wckvkr).astype(NBF)
    wq_pm = _pack_pm(wq_full).astype(NBF)

    if "nc" not in _CACHE:
        _CACHE["nc"] = _build_nc()
    nc = _CACHE["nc"]

    in_maps = []
    for core in range(NCORES):
        bo, wo = divmod(core, NW)
        heads = range(HO * core, HO * (core + 1))
        wkv_core = np.concatenate(
            [wkn[:, h * NOPE : (h + 1) * NOPE] for h in heads]
            + [wv[:, h * VD : (h + 1) * VD] for h in heads],
            axis=1,
        )  # [KVR, 256]
        wproj_core = np.concatenate(
            [w_proj[h * VD : (h + 1) * VD, :] for h in heads], axis=0
        )  # [128, D]
        ws = slice(W * wo, W * (wo + 1))
        in_maps.append(
            {
                "xTw": _pack_pm(np.ascontiguousarray(x[bo].T[:, ws])).astype(NBF),
                "cropew": np.ascontiguousarray(crope[0:ROPE, ws]),
                "cropeq": np.ascontiguousarray(crope[:, ws]),
                "wcq": wcq_pm,
                "wckvkr": wckvkr_pm,
                "wq": wq_pm,
                "wkv": _pack_pm(wkv_core).astype(NBF),
                "wproj": wproj_core.astype(NBF),
                "masks": masks,
            }
        )

    res = run_bass_kernel_spmd(nc, in_maps, list(range(NCORES)))
    LAST_RESULT = res
    outs = [np.asarray(r["out"], np.float32) for r in res.results]
    out = sum(outs)
    return out
